# revision 19
# baseline (speedup 1.0000x reference)
"""DCellLinear batched-GEMM kernel for 8 TRN2 NeuronCores.

Problem: y[s] = x[s] @ W[s].T + b[s] for 4096 independent subsystems,
x[s]: [64, 128], W[s]: [128, 128] (torch Linear layout), b[s]: [128].
Output: concat over s -> [262144, 128] float32.

Strategy (pure data parallel, no collectives):
  - Shard the subsystem axis across 8 cores (512 subsystems/core).
  - Per core, process chunks of CH=32 subsystems:
      * SWDGE cast-DMA loads f32->bf16 into a partition-contiguous layout
        (each SBUF partition holds consecutive DRAM rows -> 1 descriptor
        per partition, line-rate DMA; measured ~440 GB/s HBM-side).
      * x^T / W^T produced by REGULAR matmuls against the identity
        (out = in.T @ I into f32 PSUM): unlike PE transpose-mode (which
        pays a ~275ns non-pipelined PE-SBUF latency per instruction,
        measured +112us/pass), these pipeline at N cols/cycle and keep
        the HAM clock gate warm. Four transposes share one PSUM bank
        (one accumulation group); ONE wide DVE/ACT copy per bank drains
        it to SBUF, casting back to bf16.
      * Main matmuls in bf16, column-tiled: per subsystem pair, two
        concurrent M=64 matmuls (tile_position cols 0/64) stream each
        subsystem's W^T against its own x^T -- every PSUM element is
        useful output (no diagonal waste). Four pairs (8 subsystems)
        fill one PSUM bank [128, 4, 128] f32. On HW, start=True clears
        has_written for the written partitions only, so each column
        group opens its own accumulation; CoreSim's bank-granular group
        check mishandles base_partition=64, hence skip_group_check on
        the upper half and the bias.
      * Bias added with one K=2 matmul per bank: lhsT = e01 (partition
        0 selects output rows 0-63, partition 1 rows 64-127), rhs = the
        bank's 8 bias rows in (even, odd) layout.
      * ONE copy per bank extracts PSUM->SBUF (f32 -> bf16); HWDGE
        stores bf16 output (halves HBM write traffic; host upcasts to
        f32; max rel err 3.8e-3 vs the 2e-2 gate).
  - Accumulation stays f32 in PSUM throughout.
"""

import numpy as np
from contextlib import ExitStack

import concourse.bass as bass
import concourse.mybir as mybir
from concourse.tile import TileContext
from concourse.bass_utils import run_bass_kernel_spmd

# Problem shape (hardcoded per harness contract).
N_SUB, BATCH, D_IN, D_OUT = 4096, 64, 128, 128
N_CORES = 8
S_CORE = N_SUB // N_CORES          # 512 subsystems per core
CH = 32                            # subsystems per chunk
NCHUNK = S_CORE // CH              # 16 chunks
XR = CH * BATCH                    # 2048 x/y rows per chunk
WR = CH * D_OUT                    # 4096 W rows per chunk
XPP = XR // 128                    # 16 x-rows per SBUF partition
WPP = WR // 128                    # 32 W-rows per SBUF partition
NPAIR = CH // 2                    # 16 subsystem pairs per chunk
PAIRS_PER_BANK = 4                 # pairs sharing one PSUM y-bank
NYBANK = NPAIR // PAIRS_PER_BANK   # 4 y-banks per chunk
TSLOT = 8                          # transpose slots per PSUM bank (2KB bf16)

COMPUTE_DTYPE = mybir.dt.bfloat16


def build_nc(cdt=COMPUTE_DTYPE, hw_passes=1, sbuf_bufs=3, psum_t_bufs=6,
             psum_y_bufs=2, split_waits=True, y_bf16=True,
             cb_pattern=("dve", "dve", "act"), ex_pattern=("dve",),
             parts="all", mm_transpose=True, tslot=None):
    """hw_passes>1 wraps the whole workload in a hardware For_i loop
    (same inputs, same outputs) -- used only for timing via slope;
    results identical. cb_pattern/ex_pattern route the transpose
    copy-back / extraction copies round-robin over DVE ("dve") and ACT
    ("act"); ACT copies are slower on HW so DVE gets the bigger share,
    and extraction stays on DVE because it gates PSUM y-bank recycling."""
    nc = bass.Bass()
    ydt = cdt if y_bf16 else mybir.dt.float32
    x_in = nc.declare_dram_parameter(
        "x", [S_CORE * BATCH, D_IN], mybir.dt.float32, isOutput=False)
    w_in = nc.declare_dram_parameter(
        "W", [S_CORE * D_OUT, D_IN], mybir.dt.float32, isOutput=False)
    b_in = nc.declare_dram_parameter(
        "b", [S_CORE, D_OUT], mybir.dt.float32, isOutput=False)
    id_in = nc.declare_dram_parameter(
        "ident", [128, 128], mybir.dt.float32, isOutput=False)
    e01_in = nc.declare_dram_parameter(
        "e01", [2, 128], mybir.dt.float32, isOutput=False)
    y_out = nc.declare_dram_parameter(
        "out", [S_CORE * BATCH, D_OUT], ydt, isOutput=True)

    # Casting f32->bf16 during DMA requires SWDGE (gpsimd).
    ld = nc.gpsimd

    idx = {"cb": 0, "ex": 0}

    def copy(kind, dst, src):
        pat = cb_pattern if kind == "cb" else ex_pattern
        eng = pat[idx[kind] % len(pat)]
        idx[kind] += 1
        if eng == "act":
            nc.scalar.copy(dst, src)
        else:
            nc.vector.tensor_copy(dst, src)

    with TileContext(nc) as tc, ExitStack() as ctx:
        consts = ctx.enter_context(tc.tile_pool(name="consts", bufs=1))
        xn_pool = ctx.enter_context(tc.tile_pool(name="xn_pool", bufs=sbuf_bufs))
        wn_pool = ctx.enter_context(tc.tile_pool(name="wn_pool", bufs=sbuf_bufs))
        bc_pool = ctx.enter_context(tc.tile_pool(name="bc_pool", bufs=sbuf_bufs))
        xt_pool = ctx.enter_context(tc.tile_pool(name="xt_pool", bufs=sbuf_bufs))
        wt_pool = ctx.enter_context(tc.tile_pool(name="wt_pool", bufs=sbuf_bufs))
        yc_pool = ctx.enter_context(tc.tile_pool(name="yc_pool", bufs=sbuf_bufs))
        pt_pool = ctx.enter_context(tc.tile_pool(name="pt_pool", bufs=psum_t_bufs, space="PSUM"))
        py_pool = ctx.enter_context(tc.tile_pool(name="py_pool", bufs=psum_y_bufs, space="PSUM"))

        ident = consts.tile([128, 128], cdt)
        ld.dma_start(out=ident, in_=id_in[:, :])
        e01 = consts.tile([2, 128], cdt)
        ld.dma_start(out=e01, in_=e01_in[:, :])

        def transpose_bank(dst2, src3, t0, nslot, rpp):
            """Transpose src3[:, t0+j, :] for j in range(nslot) through one
            PSUM bank (single accumulation group), then copy all out in one
            op into the row-indexed layout dst2[i, r] = row(r)[i], where
            row r lives at source partition r // rpp, slot r % rpp.
            mm_transpose uses a regular matmul against the identity
            (out = in.T @ I accumulated in f32 PSUM, cast back to cdt in
            the copy): identical math, but pipelines at N cols/cycle and
            counts as PE-busy for the HAM clock gate, where transpose-mode
            pays a ~275ns non-pipelined PE-SBUF latency per instruction."""
            pdt = mybir.dt.float32 if mm_transpose else cdt
            ps = pt_pool.tile([128, nslot, 128], pdt)
            # A PSUM accumulation group cannot span banks: restart the group
            # at each 2KB bank boundary within the tile (slot = 128 elems).
            per_bank = 2048 // (128 * mybir.dt.size(pdt))
            for j in range(nslot):
                nc.tensor.matmul(ps[:, j, :], src3[:, t0 + j, :], ident,
                                 is_transpose=not mm_transpose,
                                 start=(j % per_bank == 0),
                                 stop=(j % per_bank == per_bank - 1 or j == nslot - 1))
            # dst positions r = rpp*p + (t0+j): strided free AP.
            dst = dst2.rearrange("i (p t) -> i t p", t=rpp)[:, t0:t0 + nslot, :]
            copy("cb", dst, ps)

        def chunk_body(c):
            # xn[p, r, i] = x_row(c*XR + XPP*p + r)[i]: per-partition data is
            # contiguous in DRAM (XPP rows of 512B).
            xn = xn_pool.tile([128, XPP, 128], cdt)
            ld.dma_start(
                out=xn,
                in_=x_in[c * XR:(c + 1) * XR, :].rearrange("(p r) i -> p r i", p=128))
            wn = wn_pool.tile([128, WPP, 128], cdt)
            ld.dma_start(
                out=wn,
                in_=w_in[c * WR:(c + 1) * WR, :].rearrange("(p r) i -> p r i", p=128))
            # bc[k, s2*128 + o] = b[c*CH + 2*s2 + k, o]  (even/odd split)
            bc = bc_pool.tile([2, NPAIR * 128], cdt)
            b_rows = b_in[:, :].rearrange("(c s2 two) o -> c two s2 o",
                                          c=NCHUNK, two=2)
            ld.dma_start(out=bc.rearrange("k (s o) -> k s o", o=128),
                         in_=b_rows[c, :, :, :])

            if parts == "loads":
                return
            ts = tslot or (4 if mm_transpose else TSLOT)
            # xt[i, r] = x_row(c*XR + r)[i]  (row-indexed transpose of x)
            xt = xt_pool.tile([128, XR], cdt)
            for t in range(0, XPP, ts):
                transpose_bank(xt, xn, t, ts, XPP)
            # wt[i, r] = W_row(c*WR + r)[i]
            wt = wt_pool.tile([128, WR], cdt)
            for t in range(0, WPP, ts):
                transpose_bank(wt, wn, t, ts, WPP)

            # yc[p, g, o] = y row (c*XR + 128g + p), col o
            yc = yc_pool.tile([128, NPAIR, 128], ydt)
            if parts == "transp":
                return
            for h in range(NYBANK):       # 4 pairs (8 subsystems) per bank
                yp = py_pool.tile([128, PAIRS_PER_BANK, 128], mybir.dt.float32)
                for q in range(PAIRS_PER_BANK):
                    g = PAIRS_PER_BANK * h + q   # pair index within chunk
                    # Column-tiled: subsystem 2g on output partitions 0-63
                    # (array cols 0-63), 2g+1 on 64-127. The two matmuls run
                    # concurrently in the PE array (separate col groups).
                    # HW start=True clears has_written for the WRITTEN
                    # partitions only (probe-verified), so each column group
                    # starts its own half; the sim's group check mishandles
                    # base_partition=64, hence skip_group_check on the B half
                    # and on the bias (the A half carries the sim group).
                    nc.tensor.matmul(yp[0:64, q, :],
                                     xt[:, 128 * g:128 * g + 64],
                                     wt[:, 256 * g:256 * g + 128],
                                     start=(q == 0),
                                     stop=(q == PAIRS_PER_BANK - 1))
                    nc.tensor.matmul(yp[64:128, q, :],
                                     xt[:, 128 * g + 64:128 * g + 128],
                                     wt[:, 256 * g + 128:256 * g + 256],
                                     start=(q == 0),
                                     stop=(q == PAIRS_PER_BANK - 1),
                                     skip_group_check=True)
                # Bias for the bank's 8 subsystems in one K=2 matmul:
                # out[m, (q, o)] += b[8h + 2q + (m>=64), o].
                nc.tensor.matmul(yp[:, :, :], e01,
                                 bc[:, h * 512:(h + 1) * 512],
                                 start=False, stop=True, skip_group_check=True)
                copy("ex", yc[:, PAIRS_PER_BANK * h:PAIRS_PER_BANK * (h + 1), :], yp)

            nc.sync.dma_start(
                out=y_out[c * XR:(c + 1) * XR, :].rearrange("(g p) o -> p g o", p=128),
                in_=yc)

        def store_only(c):
            yc = yc_pool.tile([128, NPAIR, 128], ydt)
            nc.vector.memset(yc, 0.0)
            nc.sync.dma_start(
                out=y_out[c * XR:(c + 1) * XR, :].rearrange("(g p) o -> p g o", p=128),
                in_=yc)

        # Unrolled pass loop (tc.For_i hardware loops fail codegen in this
        # walrus build: visitInstISA INTERNAL_ERROR); used for slope timing.
        for _ in range(hw_passes):
            for c in range(NCHUNK):
                if parts == "stores":
                    store_only(c)
                else:
                    chunk_body(c)

    if split_waits:
        _split_excess_waits(nc)
    return nc


# Walrus codegen allows only one sync-wait slot on engine-compute
# instructions (e.g. "Matmult: Too many sync wait commands"), but Tile's
# scheduler can emit several. Hoist the extras onto same-engine NoOps
# inserted just before the instruction: the NX sequencer processes waits
# in order before dispatch, so ordering semantics are preserved.
_WAIT_EXEMPT = {
    "InstCall", "InstUnconditionalBranch",
    "InstEventSemaphore", "InstISA", "InstHalt",
}


def _split_excess_waits(nc, max_waits=1):
    import concourse.mybir as mybir_
    k = 0
    for f in nc.m.functions:
        for blk in f.blocks:
            out = []
            changed = False
            for inst in blk.instructions:
                si = getattr(inst, "sync_info", None)
                if (si is not None and si.on_wait and len(si.on_wait) > max_waits
                        and type(inst).__name__ not in _WAIT_EXEMPT):
                    waits = list(si.on_wait)
                    for w in waits[:-max_waits]:
                        nop = mybir_.InstNoOp(name=f"I-nopw{k}")
                        k += 1
                        nop.engine = inst.engine
                        nop.sync_info = mybir_.SyncInfo(on_wait=[w], on_update=[])
                        out.append(nop)
                    inst.sync_info = mybir_.SyncInfo(
                        on_wait=waits[-max_waits:], on_update=list(si.on_update))
                    changed = True
                out.append(inst)
            if changed:
                blk.instructions = out


_CACHE = {}


def _get_nc():
    if "nc" not in _CACHE:
        _CACHE["nc"] = build_nc()
    return _CACHE["nc"]


def _constants():
    ident = np.eye(128, dtype=np.float32)
    e01 = np.zeros((2, 128), dtype=np.float32)
    e01[0, 0:64] = 1.0
    e01[1, 64:128] = 1.0
    return ident, e01


def _in_maps(x, W, b):
    ident, e01 = _constants()
    maps = []
    for i in range(N_CORES):
        sl = slice(i * S_CORE, (i + 1) * S_CORE)
        maps.append({
            "x": np.ascontiguousarray(x[sl]).reshape(S_CORE * BATCH, D_IN),
            "W": np.ascontiguousarray(W[sl]).reshape(S_CORE * D_OUT, D_IN),
            "b": np.ascontiguousarray(b[sl]),
            "ident": ident,
            "e01": e01,
        })
    return maps


def _run(x, W, b, trace=False, **kw):
    x = np.asarray(x, dtype=np.float32)
    W = np.asarray(W, dtype=np.float32)
    b = np.asarray(b, dtype=np.float32)
    res = run_bass_kernel_spmd(
        _get_nc(), _in_maps(x, W, b), core_ids=list(range(N_CORES)),
        trace=trace, **kw)
    y = np.concatenate([res.results[i]["out"] for i in range(N_CORES)], axis=0)
    return y.astype(np.float32, copy=False), res


def kernel(x, W, b):
    y, _ = _run(x, W, b, trace=False)
    return y
